# revision 1
# baseline (speedup 1.0000x reference)
"""Trainium2 Bass kernel for a BCE-based decoding loss.

Math: the reference computes, with t = tanh(llrs/2),
  p[b,r]   = clip(prod_w t[b, idx[r,w]], -1+eps, 1-eps)
  bce(z,y) = softplus(z) - z*y  with  z = -2*arctanh(p)
which for y in {0,1} simplifies exactly to
  bce = log(2) - log(1 + (1-2y) * p)
so   loss = 0.5*(M+K)*log(2) - (0.5/B) * sum_{b,r} log(1 + s[b,r]*p[b,r])
with s = 1-2y.

Sharding: pure data parallel over batch -- 8 cores x 128 rows each.

Host-side prep (data movement only, no arithmetic): llrs are cast to
bf16 and gathered per (check, w) slot into G[b, j] = llr[b, idx[j]],
ordered so the on-device product tree multiplies contiguous halves.
The label s = (1-2y) is folded into the SIGN BIT of the w=0 slot
(tanh is odd, so the device's product comes out as s*p exactly).
This is done because every data-dependent gather primitive is either
unavailable on this backend (SWDGE dma_gather, indirect_copy) or far
too slow (gpsimd ap_gather, ~27ns/idx).

Device (all the arithmetic): per 8192-slot tile,
  T = tanh(0.5*G)                      (ACT)
  tree: h=T[:4096]*T[4096:]; ... -> sp (DVE bf16 2x halving tree)
  u = max(sp, -1+eps) + 1.0            (DVE, f32)
  pm = prod over groups of 32          (DVE reduce-mult)
then one ln over all pm columns with accum_out giving the per-partition
sum of ln(1+sp); host applies the constant and the mean.
"""

import math
import os

import numpy as np

os.environ.setdefault("MYCRO_LOCAL_CACHE", "1")

import ml_dtypes  # noqa: E402

B, N, M, K = 1024, 16384, 8192, 8
WC, WO = 8, 128
NCORES = 8
BL = B // NCORES            # batch rows per core = 128
TILE_CHECKS = 512
NTILES = M // TILE_CHECKS   # 16 tiles
TILE_SLOTS = TILE_CHECKS * WC        # 8192 slots per tile
OBS_SLOTS = K * WO                   # 1024 obs slots
TOT_SLOTS = M * WC + OBS_SLOTS       # 66560
GRP = 32                             # product-group size for ln batching
EPS = 1e-6

_CACHE = {}


def build_nc():
    import concourse.bacc as bacc
    import concourse.mybir as mybir
    import concourse.tile as tile
    from contextlib import ExitStack

    nc = bacc.Bacc("TRN2", target_bir_lowering=False, debug=False)
    f32 = mybir.dt.float32
    bf16 = mybir.dt.bfloat16

    g_dram = nc.dram_tensor("g", [BL, TOT_SLOTS], bf16, kind="ExternalInput")
    out = nc.dram_tensor("out", [128, 1], f32, kind="ExternalOutput")

    Tanh = mybir.ActivationFunctionType.Tanh
    Ln = mybir.ActivationFunctionType.Ln
    PM_COLS = NTILES * (TILE_CHECKS // GRP) + 1          # 8*32 + 1 = 257

    with tile.TileContext(nc) as tc:
        with ExitStack() as ctx:
            singles = ctx.enter_context(tc.tile_pool(name="singles", bufs=1))
            gp = ctx.enter_context(tc.tile_pool(name="gp", bufs=6))
            tp = ctx.enter_context(tc.tile_pool(name="tp", bufs=3))
            tr = ctx.enter_context(tc.tile_pool(name="tr", bufs=2))

            pm = singles.tile([128, PM_COLS], f32)
            lnout = singles.tile([128, PM_COLS], f32)
            acc = singles.tile([128, 1], f32)

            for t in range(NTILES):
                g = gp.tile([128, TILE_SLOTS], bf16, tag="g")
                nc.sync.dma_start(
                    g[:], g_dram[:, t * TILE_SLOTS:(t + 1) * TILE_SLOTS])
                th = tp.tile([128, TILE_SLOTS], bf16, tag="th")
                nc.scalar.activation(th[:], g[:], Tanh, bias=0.0, scale=0.5)
                w = TILE_SLOTS
                h = th
                while w > TILE_CHECKS:
                    w //= 2
                    nh = tr.tile([128, w], bf16, tag=f"h{w}")
                    nc.vector.tensor_mul(nh[:], h[:, 0:w], h[:, w:2 * w])
                    h = nh
                u = tr.tile([128, TILE_CHECKS], f32, tag="u")
                nc.vector.tensor_scalar(
                    u[:], h[:], -(1.0 - EPS), 1.0,
                    op0=mybir.AluOpType.max, op1=mybir.AluOpType.add)
                # grouped products via 5 more tree rounds (group membership
                # is arbitrary: final[k] = prod_{j mod 32 == k} u[j])
                w = TILE_CHECKS
                h = u
                while w > TILE_CHECKS // GRP:
                    w //= 2
                    dst = (pm[:, t * (TILE_CHECKS // GRP):
                              (t + 1) * (TILE_CHECKS // GRP)]
                           if w == TILE_CHECKS // GRP
                           else tr.tile([128, w], f32, tag=f"u{w}"))
                    nc.vector.tensor_mul(dst[:], h[:, 0:w], h[:, w:2 * w])
                    h = dst

            # observables tile: 1024 slots -> tree to 8 -> u -> prod-of-8
            gob = gp.tile([128, OBS_SLOTS], bf16, tag="gob")
            nc.sync.dma_start(gob[:], g_dram[:, M * WC:TOT_SLOTS])
            tob = tp.tile([128, OBS_SLOTS], bf16, tag="tob")
            nc.scalar.activation(tob[:], gob[:], Tanh, bias=0.0, scale=0.5)
            w = OBS_SLOTS
            h = tob
            while w > K:
                w //= 2
                nh = tr.tile([128, w], bf16, tag=f"ho{w}")
                nc.vector.tensor_mul(nh[:], h[:, 0:w], h[:, w:2 * w])
                h = nh
            uo = tr.tile([128, K], f32, tag="uo")
            nc.vector.tensor_scalar(
                uo[:], h[:], -(1.0 - EPS), 1.0,
                op0=mybir.AluOpType.max, op1=mybir.AluOpType.add)
            w = K
            h = uo
            while w > 1:
                w //= 2
                dst = (pm[:, PM_COLS - 1:PM_COLS] if w == 1
                       else tr.tile([128, w], f32, tag=f"uo{w}"))
                nc.vector.tensor_mul(dst[:], h[:, 0:w], h[:, w:2 * w])
                h = dst

            # one ln over every product column; accum_out = per-partition sum
            nc.scalar.activation(
                lnout[:], pm[:], Ln, bias=0.0, scale=1.0,
                accum_out=acc[:, 0:1])

            nc.sync.dma_start(out[:, :], acc[:])

    nc.compile()
    return nc


def get_nc():
    if "nc" not in _CACHE:
        _CACHE["nc"] = build_nc()
    return _CACHE["nc"]


def build_slots(chk_idx, obs_idx):
    """Slot order: tile t, slot j = w*TILE_CHECKS + c  ->  chk[t*TC + c, w];
    then obs slots j = w*K + k -> obs[k, w].  Halving-tree pairs are
    contiguous halves at every level."""
    chk = np.asarray(chk_idx)
    obs = np.asarray(obs_idx)
    parts = []
    for t in range(NTILES):
        sub = chk[t * TILE_CHECKS:(t + 1) * TILE_CHECKS]     # [TC, WC]
        parts.append(sub.T.reshape(-1))                      # w-major
    parts.append(obs.T.reshape(-1))                          # [WO*K]
    return np.concatenate(parts).astype(np.int64)


def make_in_maps(llrs, syndromes, observables, chk_idx, obs_idx):
    llr_bf = np.asarray(llrs).astype(ml_dtypes.bfloat16)
    slots = build_slots(chk_idx, obs_idx)
    g_all = np.take(llr_bf, slots, axis=1)                   # [B, TOT_SLOTS]
    # fold s = (1-2y) into the sign bit of the w=0 slot of each check
    v = g_all.view(np.uint16)
    syn = np.asarray(syndromes)
    for t in range(NTILES):
        cols = slice(t * TILE_SLOTS, t * TILE_SLOTS + TILE_CHECKS)
        ycols = slice(t * TILE_CHECKS, (t + 1) * TILE_CHECKS)
        v[:, cols] ^= (syn[:, ycols] != 0).astype(np.uint16) << 15
    yobs = (np.asarray(observables) != 0).astype(np.uint16) << 15
    v[:, M * WC:M * WC + K] ^= yobs
    return [{"g": g_all[BL * c:BL * (c + 1)]} for c in range(NCORES)]


def finish(results):
    total = 0.0
    for r in results:
        total += float(np.asarray(r["out"]).astype(np.float64).sum())
    loss = 0.5 * (M + K) * math.log(2.0) - 0.5 * total / B
    return np.float32(loss)


def kernel(llrs, syndromes, observables, chk_idx, obs_idx):
    from concourse.bass_utils import run_bass_kernel_spmd

    in_maps = make_in_maps(llrs, syndromes, observables, chk_idx, obs_idx)
    nc = get_nc()
    res = run_bass_kernel_spmd(nc, in_maps, core_ids=list(range(NCORES)))
    return finish(res.results)

# uniform-plan alias for the test harness model
PLAN = [TILE_CHECKS] * NTILES

